# revision 24
# baseline (speedup 1.0000x reference)
"""CfC (closed-form continuous-time RNN / NCP) kernel for 8 Trainium2 NeuronCores.

Model (see reference): 3 stacked CfC layers, scan over T=256 timesteps,
B=1024 batch, OBS=64 input features; heads: actor [B,T,8], value [B,T,1],
plus final hidden state hx [B,64].

Strategy
--------
Data-parallel over batch: 8 cores x 128 batch rows. Per core the three
layers are computed as a layer-pipelined "wavefront": at stage s the
kernel computes layer0(t=s), layer1(t=s-1), layer2(t=s-2) in the SAME
set of matmuls, because the combined state vector
    S(s) = [n_a(s); n_b(s-1); n_c(s-2)]   (20+12+32 = 64 rows)
feeds all three layers with (overlapping) contiguous K-row windows.

Feature-major layout: activations live as [feature, batch] tiles so the
PE contraction dim (K) is the feature dim. Per stage and per batch-half
(the 128 batch rows are split into two independent 64-row recurrences so
their serial chains interleave across engines):
  rhs RS(s) [128,64] = [S(s-1) (rows 0:64) ; x(s).T (rows 64:128)]
  mm_bias (K=3)  : PSUM[64,192] = biases (start=True broadcast matmul)
  mm_ff1/ff2/t   : PSUM[:,c:c+64] += W.T @ RS  (t-weights half-scaled)
  ACT            : FT = tanh(PSUM)   (sigmoid(a) = (1+tanh(a/2))/2)
  GPSIMD         : d = ff2 - ff1
  DVE            : m = (u+1)*d ; h = 0.5*m + ff1 -> RS(s+1)[0:64]
  GPSIMD         : y = relu(n_c rows of h) -> head staging
  PE (per 3 t's) : head matmul  out[64,27] = Y3.T @ AW  (batch-major)
The head output is batch-major so no transposes are needed except a
single 64x64 PE transpose per half for hx at the end.

Host side transposes x to [T, 2, 64, 64] per core (contiguous per-stage
slices) and gathers/reshapes the outputs.
"""

import numpy as np

OBS, B, T = 64, 1024, 256
SIZES = [(20, 84), (12, 32), (32, 44)]
IN_FEATS = [OBS, 20, 12]
NCORES = 8
BSH = B // NCORES   # 128 batch rows per core
HB = BSH // 2       # 64 batch rows per half
NSTAGES = T + 2     # wavefront ramp: layer2 finishes t=T-1 at stage T+1
NGROUPS = (T + 2) // 3  # 86 head groups of up to 3 timesteps

# S-row layout offsets
H_OFF = [0, 20, 32]       # h-block (recurrent state) offset per layer
IN_OFF = [None, 0, 20]    # input-block offset inside S (layers 1,2)
M_OFF = [0, 20, 32]       # ff1 output row offset per layer (ff2 at +64)


def _build_consts(inp):
    """Build the fused weight matrices (lhsT layouts) on the host.

    DVE tensor_tensor ops require both SBUF operands at the same base
    partition, so ff1 / ff2 / u all live in rows 0:64 of the PSUM/FT tile,
    side by side in columns: cols 0:HB = ff1, HB:2HB = ff2, 2HB:3HB = u.
    """
    f32 = np.float32
    WA = np.zeros((128, 128), f32)   # lhsT: cols 0:64 -> ff1, 64:128 -> ff2
    WB = np.zeros((128, 64), f32)    # lhsT for t preacts (half-scaled)
    bA = np.zeros(128, f32)
    bB = np.zeros(64, f32)
    for l, (h, c) in enumerate(SIZES):
        w1 = inp[f"w1_{l}"] * inp[f"mask_{l}"]
        w2 = inp[f"w2_{l}"] * inp[f"mask_{l}"]
        wt = (inp[f"ta_w_{l}"] + inp[f"tb_w_{l}"]) * 0.5
        nin = IN_FEATS[l]
        mo = M_OFF[l]
        ho = H_OFF[l]
        if l == 0:
            # x part -> K rows 64:128, h part -> K rows 0:20
            WA[64:64 + nin, mo:mo + h] = w1[:, :nin].T
            WA[64:64 + nin, 64 + mo:64 + mo + h] = w2[:, :nin].T
            WA[ho:ho + 20, mo:mo + h] = w1[:, nin:].T
            WA[ho:ho + 20, 64 + mo:64 + mo + h] = w2[:, nin:].T
            WB[64:64 + nin, mo:mo + h] = wt[:, :nin].T
            WB[ho:ho + 20, mo:mo + h] = wt[:, nin:].T
        else:
            io = IN_OFF[l]
            WA[io:io + nin, mo:mo + h] = w1[:, :nin].T
            WA[io:io + nin, 64 + mo:64 + mo + h] = w2[:, :nin].T
            WA[ho:ho + h, mo:mo + h] = w1[:, nin:].T
            WA[ho:ho + h, 64 + mo:64 + mo + h] = w2[:, nin:].T
            WB[io:io + nin, mo:mo + h] = wt[:, :nin].T
            WB[ho:ho + h, mo:mo + h] = wt[:, nin:].T
        bA[mo:mo + h] = inp[f"b1_{l}"]
        bA[64 + mo:64 + mo + h] = inp[f"b2_{l}"]
        bB[mo:mo + h] = (inp[f"ta_b_{l}"] + inp[f"tb_b_{l}"]) * 0.5

    # bias matmul: PSUM[64,3*HB] = WBIAS.T @ BSEL, WBIAS [3,64], BSEL [3,3*HB]
    WBIAS = np.stack([bA[0:64], bA[64:128], bB], axis=0)   # [3, 64]
    BSEL = np.zeros((3, 3 * HB), f32)
    BSEL[0, 0:HB] = 1.0
    BSEL[1, HB:2 * HB] = 1.0
    BSEL[2, 2 * HB:3 * HB] = 1.0

    # head rhs AW [99, 27]: K rows = [y(j=0) 0:32 | y(j=1) 32:64 | y(j=2)
    # 64:96 | ones rows 96:99] -> 3 x (8 actor + 1 value) output cols.
    # (y blocks are 32-partition aligned because GPSIMD requires it.)
    aw, ab = inp["action_w"], inp["action_b"]       # [8,32], [8]
    vw, vb = inp["value_w"], inp["value_b"]         # [1,32], [1]
    AW = np.zeros((99, 27), f32)
    for j in range(3):
        AW[32 * j:32 * j + 32, 9 * j:9 * j + 8] = aw.T
        AW[32 * j:32 * j + 32, 9 * j + 8:9 * j + 9] = vw.T
        AW[96 + j, 9 * j:9 * j + 8] = ab
        AW[96 + j, 9 * j + 8] = vb[0]
    return WA, WB, WBIAS, BSEL, AW


def _build_program():
    import concourse.bass as bass
    import concourse.mybir as mybir
    import concourse.tile as tile
    from concourse import bacc
    from concourse.masks import make_identity

    f32 = mybir.dt.float32
    AF = mybir.ActivationFunctionType
    OP = mybir.AluOpType

    nc = bacc.Bacc()

    xt_d = nc.declare_dram_parameter("xt", [T, 2, 64, HB], f32, isOutput=False)
    h0t_d = nc.declare_dram_parameter("h0t", [2, 64, HB], f32, isOutput=False)
    wa_d = nc.declare_dram_parameter("wa", [128, 128], f32, isOutput=False)
    wb_d = nc.declare_dram_parameter("wb", [128, 64], f32, isOutput=False)
    wbias_d = nc.declare_dram_parameter("wbias", [3, 64], f32, isOutput=False)
    bsel_d = nc.declare_dram_parameter("bsel", [3, 3 * HB], f32, isOutput=False)
    aw_d = nc.declare_dram_parameter("aw", [99, 27], f32, isOutput=False)

    actor_d = nc.declare_dram_parameter("actor", [BSH, T * 8], f32, isOutput=True)
    value_d = nc.declare_dram_parameter("value", [BSH, T], f32, isOutput=True)
    hx_d = nc.declare_dram_parameter("hx", [BSH, 64], f32, isOutput=True)

    with tile.TileContext(nc) as tc:
        with (
            tc.tile_pool(name="consts", bufs=1) as cpool,
            tc.tile_pool(name="persist", bufs=1) as ppool,
            tc.tile_pool(name="rs", bufs=10) as rs_pool,
            tc.tile_pool(name="ft", bufs=6) as ft_pool,
            tc.tile_pool(name="dm", bufs=6) as dm_pool,
            tc.tile_pool(name="y3", bufs=4) as y3_pool,
            tc.tile_pool(name="psum", bufs=3, space="PSUM") as ps_pool,
            tc.tile_pool(name="psum_h", bufs=1, space="PSUM") as psh_pool,
        ):
            # ---- constants ----
            WA_t = cpool.tile([128, 128], f32, tag="wa")
            nc.sync.dma_start(WA_t[:], wa_d[:])
            WB_t = cpool.tile([128, 64], f32, tag="wb")
            nc.sync.dma_start(WB_t[:], wb_d[:])
            WBIAS_t = cpool.tile([3, 64], f32, tag="wbias")
            nc.sync.dma_start(WBIAS_t[:], wbias_d[:])
            BSEL_t = cpool.tile([3, 3 * HB], f32, tag="bsel")
            nc.sync.dma_start(BSEL_t[:], bsel_d[:])
            AW_t = cpool.tile([99, 27], f32, tag="aw")
            nc.sync.dma_start(AW_t[:], aw_d[:])
            ID64 = cpool.tile([64, 64], f32, tag="id64")
            make_identity(nc, ID64[:])

            ABUF9 = ppool.tile([128, 27 * NGROUPS], f32, tag="abuf9")
            ABUF = ppool.tile([128, T * 8], f32, tag="abuf")
            VBUF = ppool.tile([128, T], f32, tag="vbuf")

            # ---- stage 0 rhs (per half) ----
            rs_cur = []
            for hf in range(2):
                t0 = rs_pool.tile([128, HB], f32, tag=f"rs{hf}")
                nc.sync.dma_start(t0[0:64, :], h0t_d[hf])
                nc.sync.dma_start(t0[64:128, :], xt_d[0, hf])
                rs_cur.append(t0)

            y3 = [None, None]
            rs_hist = {}  # (stage, hf) handles for the hx assembly

            for s in range(NSTAGES):
                rs_next = []
                for hf in range(2):
                    rn = rs_pool.tile([128, HB], f32, tag=f"rs{hf}")
                    if s + 1 < T:
                        nc.sync.dma_start(rn[64:128, :], xt_d[s + 1, hf])
                    elif s + 1 <= NSTAGES:
                        nc.gpsimd.memset(rn[64:96, :], 0.0)
                        nc.gpsimd.memset(rn[96:128, :], 0.0)
                    rs_next.append(rn)

                for hf in range(2):
                    rsc, rsn = rs_cur[hf], rs_next[hf]
                    # matmuls: bias broadcast then accumulate ff1 / ff2 / t
                    pab = ps_pool.tile([64, 3 * HB], f32, tag=f"pab{hf}")
                    nc.tensor.matmul(pab[:, :], WBIAS_t[:], BSEL_t[:],
                                     start=True, stop=False,
                                     skip_group_check=True)
                    nc.tensor.matmul(pab[:, 0:HB], WA_t[:, 0:64], rsc[:],
                                     start=False, stop=True,
                                     skip_group_check=True)
                    nc.tensor.matmul(pab[:, HB:2 * HB], WA_t[:, 64:128],
                                     rsc[:], start=False, stop=True,
                                     skip_group_check=True)
                    nc.tensor.matmul(pab[:, 2 * HB:3 * HB], WB_t[:], rsc[:],
                                     start=False, stop=True,
                                     skip_group_check=True)

                    # merged tanh: ff1 | ff2 | u (all at base partition 0)
                    ft = ft_pool.tile([64, 3 * HB], f32, tag=f"ft{hf}")
                    nc.scalar.activation(ft[:, :], pab[:, :], AF.Tanh)

                    # gating: h = ff1 + 0.5*(u+1)*(ff2-ff1)
                    d_t = dm_pool.tile([64, HB], f32, tag=f"d{hf}")
                    nc.gpsimd.tensor_sub(d_t[:], ft[:, HB:2 * HB],
                                         ft[:, 0:HB])
                    m_t = dm_pool.tile([64, HB], f32, tag=f"m{hf}")
                    nc.vector.scalar_tensor_tensor(
                        m_t[:], ft[:, 2 * HB:3 * HB], 1.0, d_t[:],
                        op0=OP.add, op1=OP.mult)
                    nc.vector.scalar_tensor_tensor(
                        rsn[0:64, :], m_t[:], 0.5, ft[:, 0:HB],
                        op0=OP.mult, op1=OP.add)

                    # wavefront ramp: restore h0 for layers not yet active
                    # (DMA: engine copies need 32-aligned partition bases)
                    if s == 0:
                        nc.sync.dma_start(rsn[20:64, :], h0t_d[hf, 20:64, :])
                    elif s == 1:
                        nc.sync.dma_start(rsn[32:64, :], h0t_d[hf, 32:64, :])

                    # head: y = relu(n_c) for t_out = s-2
                    if s >= 2:
                        t_out = s - 2
                        j = t_out % 3
                        g = t_out // 3
                        if j == 0:
                            y3[hf] = y3_pool.tile([99, HB], f32,
                                                  name=f"y3_{hf}",
                                                  tag=f"y3{hf}")
                            nc.gpsimd.memset(y3[hf][96:99, :], 1.0)
                            if t_out == T - 1:
                                # tail group: zero unused y(j=1,2) slots
                                nc.gpsimd.memset(y3[hf][32:64, :], 0.0)
                                nc.gpsimd.memset(y3[hf][64:96, :], 0.0)
                        nc.gpsimd.tensor_relu(y3[hf][32 * j:32 * j + 32, :],
                                              rsn[32:64, :])
                        if j == 2 or t_out == T - 1:
                            ph = psh_pool.tile([64, 27], f32, tag=f"ph{hf}")
                            nc.tensor.matmul(ph[:, :], y3[hf][:, :],
                                             AW_t[:, :], start=True,
                                             stop=True)
                            nc.vector.tensor_copy(
                                ABUF9[64 * hf:64 * hf + 64,
                                      27 * g:27 * g + 27],
                                ph[:, :])

                    if s >= NSTAGES - 3:
                        rs_hist[(s + 1, hf)] = rs_next[hf]
                rs_cur = rs_next

            # ---- repack head outputs ----
            # actor: t = 3g+j (t<255 full groups), col in ABUF9 = 27g+9j+o
            src_a = ABUF9[:, 0:2295].rearrange("p (g j q) -> p g j q",
                                               j=3, q=9)
            src_a = src_a[:, 0:85, :, 0:8]
            dst_a = ABUF[:, 0:2040].rearrange("p (g j o) -> p g j o",
                                              j=3, o=8)
            nc.vector.tensor_copy(dst_a, src_a)
            nc.vector.tensor_copy(ABUF[:, 2040:2048],
                                  ABUF9[:, 27 * 85:27 * 85 + 8])
            src_v = ABUF9[:, 0:2295].rearrange("p (g j q) -> p g j q",
                                               j=3, q=9)
            src_v = src_v[:, 0:85, :, 8:9]
            dst_v = VBUF[:, 0:255].rearrange("p (g j q) -> p g j q",
                                             j=3, q=1)
            nc.vector.tensor_copy(dst_v, src_v)
            nc.vector.tensor_copy(VBUF[:, 255:256],
                                  ABUF9[:, 27 * 85 + 8:27 * 85 + 9])

            # ---- hx: [n_a(T-1); n_b(T-1); n_c(T-1)] then transpose ----
            HXT = ppool.tile([128, 64], f32, tag="hxt")
            for hf in range(2):
                HX = ppool.tile([64, HB], f32, tag=f"hx{hf}")
                nc.sync.dma_start(HX[0:20, :], rs_hist[(T, hf)][0:20, :])
                nc.sync.dma_start(HX[20:32, :], rs_hist[(T + 1, hf)][20:32, :])
                nc.sync.dma_start(HX[32:64, :], rs_hist[(T + 2, hf)][32:64, :])
                ph_hx = psh_pool.tile([64, 64], f32, tag=f"ph{hf}")
                nc.tensor.transpose(ph_hx[:, :], HX[:], ID64[:])
                nc.vector.tensor_copy(HXT[64 * hf:64 * hf + 64, :],
                                      ph_hx[:, :])

            # ---- outputs ----
            nc.sync.dma_start(actor_d[:], ABUF[:])
            nc.sync.dma_start(value_d[:], VBUF[:])
            nc.sync.dma_start(hx_d[:], HXT[:])

    nc.finalize()
    return nc


_NC_CACHE = None


def _get_nc():
    global _NC_CACHE
    if _NC_CACHE is None:
        _NC_CACHE = _build_program()
    return _NC_CACHE


def kernel(trace=False, **inputs):
    from concourse.bass_utils import run_bass_kernel_spmd

    inp = {k: np.asarray(v, dtype=np.float32) for k, v in inputs.items()}
    WA, WB, WBIAS, BSEL, AW = _build_consts(inp)

    x = inp["x"]          # [B, T, OBS]
    h0 = inp["h0"]        # [B, 64]

    in_maps = []
    for c in range(NCORES):
        b0, b1 = c * BSH, (c + 1) * BSH
        # [T, 2, 64, HB]: per stage, per batch-half, feature-major slice
        xc = x[b0:b1].transpose(1, 2, 0).reshape(T, 64, 2, HB)
        xt = np.ascontiguousarray(xc.transpose(0, 2, 1, 3))
        h0c = h0[b0:b1].T.reshape(64, 2, HB)
        h0t = np.ascontiguousarray(h0c.transpose(1, 0, 2))  # [2, 64, HB]
        in_maps.append({
            "xt": xt, "h0t": h0t, "wa": WA, "wb": WB,
            "wbias": WBIAS, "bsel": BSEL, "aw": AW,
        })

    nc = _get_nc()
    res = run_bass_kernel_spmd(nc, in_maps, list(range(NCORES)), trace=trace)

    actor = np.concatenate([res.results[c]["actor"].reshape(BSH, T, 8)
                            for c in range(NCORES)], axis=0)
    value = np.concatenate([res.results[c]["value"].reshape(BSH, T, 1)
                            for c in range(NCORES)], axis=0)
    hx = np.concatenate([res.results[c]["hx"] for c in range(NCORES)], axis=0)
    kernel.last_exec_time_ns = res.exec_time_ns
    return actor, value, hx


# revision 33
# speedup vs baseline: 1.1631x; 1.1631x over previous
"""CfC (closed-form continuous-time RNN / NCP) kernel for 8 Trainium2 NeuronCores.

Model (see reference): 3 stacked CfC layers, scan over T=256 timesteps,
B=1024 batch, OBS=64 input features; heads: actor [B,T,8], value [B,T,1],
plus final hidden state hx [B,64].

Strategy
--------
Data-parallel over batch: 8 cores x 128 batch rows. Per core the three
layers are computed as a layer-pipelined "wavefront": at stage s the
kernel computes layer0(t=s), layer1(t=s-1), layer2(t=s-2) in the SAME
set of matmuls, because the combined state vector
    S(s) = [n_a(s); n_b(s-1); n_c(s-2)]   (20+12+32 = 64 rows)
feeds all three layers with (overlapping) contiguous K-row windows.

Feature-major layout: activations live as [feature, batch] tiles so the
PE contraction dim (K) is the feature dim. Per stage and per batch-half
(the 128 batch rows are split into two independent 64-row recurrences so
their serial chains interleave across engines):
  rhs RS(s) [128,64] = [S(s-1) (rows 0:64) ; x(s).T (rows 64:128)]
  mm_bias (K=3)  : PSUM[64,192] = biases (start=True broadcast matmul)
  mm_ff1/ff2/t   : PSUM[:,c:c+64] += W.T @ RS  (t-weights half-scaled)
  ACT            : FT = tanh(PSUM)   (sigmoid(a) = (1+tanh(a/2))/2)
  GPSIMD         : d = ff2 - ff1
  DVE            : m = (u+1)*d ; h = 0.5*m + ff1 -> RS(s+1)[0:64]
  GPSIMD         : y = relu(n_c rows of h) -> head staging
  PE (per 3 t's) : head matmul  out[64,27] = Y3.T @ AW  (batch-major)
The head output is batch-major so no transposes are needed except a
single 64x64 PE transpose per half for hx at the end.

Host side transposes x to [T, 2, 64, 64] per core (contiguous per-stage
slices) and gathers/reshapes the outputs.
"""

import numpy as np

OBS, B, T = 64, 1024, 256
SIZES = [(20, 84), (12, 32), (32, 44)]
IN_FEATS = [OBS, 20, 12]
NCORES = 8
BSH = B // NCORES   # 128 batch rows per core
HB = BSH // 2       # 64 batch rows per half
NSTAGES = T + 2     # wavefront ramp: layer2 finishes t=T-1 at stage T+1
NGROUPS = (T + 2) // 3  # 86 head groups of up to 3 timesteps

# S-row layout offsets
H_OFF = [0, 20, 32]       # h-block (recurrent state) offset per layer
IN_OFF = [None, 0, 20]    # input-block offset inside S (layers 1,2)
M_OFF = [0, 20, 32]       # ff1 output row offset per layer (ff2 at +64)


def _build_consts(inp):
    """Build the fused weight matrices (lhsT layouts) on the host.

    DVE tensor_tensor ops require both SBUF operands at the same base
    partition, so ff1 / ff2 / u all live in rows 0:64 of the PSUM/FT tile,
    side by side in columns: cols 0:HB = ff1, HB:2HB = ff2, 2HB:3HB = u.
    """
    f32 = np.float32
    WA = np.zeros((128, 128), f32)   # lhsT: cols 0:64 -> ff1, 64:128 -> ff2
    WB = np.zeros((128, 64), f32)    # lhsT for t preacts (half-scaled)
    bA = np.zeros(128, f32)
    bB = np.zeros(64, f32)
    for l, (h, c) in enumerate(SIZES):
        w1 = inp[f"w1_{l}"] * inp[f"mask_{l}"]
        w2 = inp[f"w2_{l}"] * inp[f"mask_{l}"]
        wt = (inp[f"ta_w_{l}"] + inp[f"tb_w_{l}"]) * 0.5
        nin = IN_FEATS[l]
        mo = M_OFF[l]
        ho = H_OFF[l]
        if l == 0:
            # x part -> K rows 64:128, h part -> K rows 0:20
            WA[64:64 + nin, mo:mo + h] = w1[:, :nin].T
            WA[64:64 + nin, 64 + mo:64 + mo + h] = w2[:, :nin].T
            WA[ho:ho + 20, mo:mo + h] = w1[:, nin:].T
            WA[ho:ho + 20, 64 + mo:64 + mo + h] = w2[:, nin:].T
            WB[64:64 + nin, mo:mo + h] = wt[:, :nin].T
            WB[ho:ho + 20, mo:mo + h] = wt[:, nin:].T
        else:
            io = IN_OFF[l]
            WA[io:io + nin, mo:mo + h] = w1[:, :nin].T
            WA[io:io + nin, 64 + mo:64 + mo + h] = w2[:, :nin].T
            WA[ho:ho + h, mo:mo + h] = w1[:, nin:].T
            WA[ho:ho + h, 64 + mo:64 + mo + h] = w2[:, nin:].T
            WB[io:io + nin, mo:mo + h] = wt[:, :nin].T
            WB[ho:ho + h, mo:mo + h] = wt[:, nin:].T
        bA[mo:mo + h] = inp[f"b1_{l}"]
        bA[64 + mo:64 + mo + h] = inp[f"b2_{l}"]
        bB[mo:mo + h] = (inp[f"ta_b_{l}"] + inp[f"tb_b_{l}"]) * 0.5

    # Doubled-state trick: the kernel stores h' = 2h (so the gating is the
    # constant-free  h' = ff1 + ff2 + u*(ff2-ff1), all plain tensor_tensor
    # ops, which GPSIMD supports).  Consumers compensate: every weight that
    # reads the state rows (K rows 0:64) is halved; x-part rows unchanged.
    WA[0:64, :] *= 0.5
    WB[0:64, :] *= 0.5

    # bias matmul: PSUM[64,3*HB] = WBIAS.T @ BSEL, WBIAS [3,64], BSEL [3,3*HB]
    WBIAS = np.stack([bA[0:64], bA[64:128], bB], axis=0)   # [3, 64]
    BSEL = np.zeros((3, 3 * HB), f32)
    BSEL[0, 0:HB] = 1.0
    BSEL[1, HB:2 * HB] = 1.0
    BSEL[2, 2 * HB:3 * HB] = 1.0

    # head rhs AW [99, 27]: K rows = [y(j=0) 0:32 | y(j=1) 32:64 | y(j=2)
    # 64:96 | ones rows 96:99] -> 3 x (8 actor + 1 value) output cols.
    # (y blocks are 32-partition aligned because GPSIMD requires it.)
    aw, ab = inp["action_w"], inp["action_b"]       # [8,32], [8]
    vw, vb = inp["value_w"], inp["value_b"]         # [1,32], [1]
    AW = np.zeros((99, 27), f32)
    for j in range(3):
        # y rows halved: y3 holds relu(2*n_c) = 2*relu(n_c)
        AW[32 * j:32 * j + 32, 9 * j:9 * j + 8] = aw.T * 0.5
        AW[32 * j:32 * j + 32, 9 * j + 8:9 * j + 9] = vw.T * 0.5
        AW[96 + j, 9 * j:9 * j + 8] = ab
        AW[96 + j, 9 * j + 8] = vb[0]
    return WA, WB, WBIAS, BSEL, AW


def _build_program():
    import concourse.bass as bass
    import concourse.mybir as mybir
    import concourse.tile as tile
    from concourse import bacc
    from concourse.masks import make_identity

    f32 = mybir.dt.float32
    AF = mybir.ActivationFunctionType
    OP = mybir.AluOpType

    nc = bacc.Bacc()

    xt_d = nc.declare_dram_parameter("xt", [T, 2, 64, HB], f32, isOutput=False)
    h0t_d = nc.declare_dram_parameter("h0t", [2, 64, HB], f32, isOutput=False)
    wa_d = nc.declare_dram_parameter("wa", [128, 128], f32, isOutput=False)
    wb_d = nc.declare_dram_parameter("wb", [128, 64], f32, isOutput=False)
    wbias_d = nc.declare_dram_parameter("wbias", [3, 64], f32, isOutput=False)
    bsel_d = nc.declare_dram_parameter("bsel", [3, 3 * HB], f32, isOutput=False)
    aw_d = nc.declare_dram_parameter("aw", [99, 27], f32, isOutput=False)

    actor_d = nc.declare_dram_parameter("actor", [BSH, T * 8], f32, isOutput=True)
    value_d = nc.declare_dram_parameter("value", [BSH, T], f32, isOutput=True)
    hx_d = nc.declare_dram_parameter("hx", [BSH, 64], f32, isOutput=True)

    with tile.TileContext(nc) as tc:
        with (
            tc.tile_pool(name="consts", bufs=1) as cpool,
            tc.tile_pool(name="persist", bufs=1) as ppool,
            tc.tile_pool(name="rs", bufs=10) as rs_pool,
            tc.tile_pool(name="ft", bufs=6) as ft_pool,
            tc.tile_pool(name="dm", bufs=6) as dm_pool,
            tc.tile_pool(name="y3", bufs=4) as y3_pool,
            tc.tile_pool(name="psum", bufs=3, space="PSUM") as ps_pool,
            tc.tile_pool(name="psum_h", bufs=1, space="PSUM") as psh_pool,
        ):
            # ---- constants ----
            WA_t = cpool.tile([128, 128], f32, tag="wa")
            nc.sync.dma_start(WA_t[:], wa_d[:])
            WB_t = cpool.tile([128, 64], f32, tag="wb")
            nc.sync.dma_start(WB_t[:], wb_d[:])
            WBIAS_t = cpool.tile([3, 64], f32, tag="wbias")
            nc.sync.dma_start(WBIAS_t[:], wbias_d[:])
            BSEL_t = cpool.tile([3, 3 * HB], f32, tag="bsel")
            nc.sync.dma_start(BSEL_t[:], bsel_d[:])
            AW_t = cpool.tile([99, 27], f32, tag="aw")
            nc.sync.dma_start(AW_t[:], aw_d[:])
            ID64 = cpool.tile([64, 64], f32, tag="id64")
            make_identity(nc, ID64[:])

            ABUF9 = ppool.tile([128, 27 * NGROUPS], f32, tag="abuf9")
            ABUF = ppool.tile([128, T * 8], f32, tag="abuf")
            VBUF = ppool.tile([128, T], f32, tag="vbuf")

            # ---- stage 0 rhs (per half) ----
            rs_cur = []
            for hf in range(2):
                t0 = rs_pool.tile([128, HB], f32, tag=f"rs{hf}")
                nc.sync.dma_start(t0[0:64, :], h0t_d[hf])
                nc.sync.dma_start(t0[64:128, :], xt_d[0, hf])
                rs_cur.append(t0)

            y3 = [None, None]
            rs_hist = {}  # (stage, hf) handles for the hx assembly

            for s in range(NSTAGES):
                rs_next = []
                for hf in range(2):
                    rn = rs_pool.tile([128, HB], f32, tag=f"rs{hf}")
                    if s + 1 < T:
                        nc.sync.dma_start(rn[64:128, :], xt_d[s + 1, hf])
                    elif s + 1 <= NSTAGES:
                        nc.gpsimd.memset(rn[64:96, :], 0.0)
                        nc.gpsimd.memset(rn[96:128, :], 0.0)
                    rs_next.append(rn)

                for hf in range(2):
                    rsc, rsn = rs_cur[hf], rs_next[hf]
                    # matmuls: bias broadcast then accumulate ff1 / ff2 / t
                    pab = ps_pool.tile([64, 3 * HB], f32, tag=f"pab{hf}")
                    nc.tensor.matmul(pab[:, :], WBIAS_t[:], BSEL_t[:],
                                     start=True, stop=False,
                                     skip_group_check=True)
                    nc.tensor.matmul(pab[:, 0:HB], WA_t[:, 0:64], rsc[:],
                                     start=False, stop=True,
                                     skip_group_check=True)
                    nc.tensor.matmul(pab[:, HB:2 * HB], WA_t[:, 64:128],
                                     rsc[:], start=False, stop=True,
                                     skip_group_check=True)
                    nc.tensor.matmul(pab[:, 2 * HB:3 * HB], WB_t[:], rsc[:],
                                     start=False, stop=True,
                                     skip_group_check=True)

                    # merged tanh: ff1 | ff2 | u (all at base partition 0)
                    ft = ft_pool.tile([64, 3 * HB], f32, tag=f"ft{hf}")
                    nc.scalar.activation(ft[:, :], pab[:, :], AF.Tanh)

                    # gating in doubled-state form (all plain TT, all on
                    # GPSIMD so the dependent ops pipeline on one engine):
                    #   h' = 2h = ff1 + ff2 + u*(ff2-ff1)
                    d_t = dm_pool.tile([64, HB], f32, tag=f"d{hf}")
                    nc.gpsimd.tensor_sub(d_t[:], ft[:, HB:2 * HB],
                                         ft[:, 0:HB])
                    s_t = dm_pool.tile([64, HB], f32, tag=f"s{hf}")
                    nc.gpsimd.tensor_add(s_t[:], ft[:, 0:HB],
                                         ft[:, HB:2 * HB])
                    p_t = dm_pool.tile([64, HB], f32, tag=f"p{hf}")
                    nc.gpsimd.tensor_mul(p_t[:], ft[:, 2 * HB:3 * HB],
                                         d_t[:])
                    nc.gpsimd.tensor_add(rsn[0:64, :], s_t[:], p_t[:])

                    # wavefront ramp: restore h0 for layers not yet active
                    # (DMA: engine copies need 32-aligned partition bases)
                    if s == 0:
                        nc.sync.dma_start(rsn[20:64, :], h0t_d[hf, 20:64, :])
                    elif s == 1:
                        nc.sync.dma_start(rsn[32:64, :], h0t_d[hf, 32:64, :])

                    # head: y = relu(n_c) for t_out = s-2; both halves share
                    # one Y3 [99,128] (halves side by side in columns) so a
                    # single head matmul covers the full 128-row batch
                    if s >= 2:
                        t_out = s - 2
                        j = t_out % 3
                        g = t_out // 3
                        if j == 0 and hf == 0:
                            y3[0] = y3_pool.tile([99, BSH], f32,
                                                 name="y3g", tag="y3")
                            nc.gpsimd.memset(y3[0][96:99, :], 1.0)
                            if t_out == T - 1:
                                # tail group: zero unused y(j=1,2) slots
                                nc.gpsimd.memset(y3[0][32:64, :], 0.0)
                                nc.gpsimd.memset(y3[0][64:96, :], 0.0)
                        # relu on DVE (Pool carries the gating chain);
                        # y3 holds 2*relu(n_c), AW y-rows are pre-halved
                        nc.vector.tensor_scalar_max(
                            y3[0][32 * j:32 * j + 32, HB * hf:HB * hf + HB],
                            rsn[32:64, :], 0.0)
                        if (j == 2 or t_out == T - 1) and hf == 1:
                            ph = psh_pool.tile([128, 27], f32, tag="ph")
                            nc.tensor.matmul(ph[:, :], y3[0][:, :],
                                             AW_t[:, :], start=True,
                                             stop=True)
                            nc.vector.tensor_copy(
                                ABUF9[:, 27 * g:27 * g + 27], ph[:, :])

                    if s >= NSTAGES - 3:
                        rs_hist[(s + 1, hf)] = rs_next[hf]
                rs_cur = rs_next

            # ---- repack head outputs ----
            # actor: t = 3g+j (t<255 full groups), col in ABUF9 = 27g+9j+o
            src_a = ABUF9[:, 0:2295].rearrange("p (g j q) -> p g j q",
                                               j=3, q=9)
            src_a = src_a[:, 0:85, :, 0:8]
            dst_a = ABUF[:, 0:2040].rearrange("p (g j o) -> p g j o",
                                              j=3, o=8)
            nc.vector.tensor_copy(dst_a, src_a)
            nc.vector.tensor_copy(ABUF[:, 2040:2048],
                                  ABUF9[:, 27 * 85:27 * 85 + 8])
            src_v = ABUF9[:, 0:2295].rearrange("p (g j q) -> p g j q",
                                               j=3, q=9)
            src_v = src_v[:, 0:85, :, 8:9]
            dst_v = VBUF[:, 0:255].rearrange("p (g j q) -> p g j q",
                                             j=3, q=1)
            nc.vector.tensor_copy(dst_v, src_v)
            nc.vector.tensor_copy(VBUF[:, 255:256],
                                  ABUF9[:, 27 * 85 + 8:27 * 85 + 9])

            # ---- hx: [n_a(T-1); n_b(T-1); n_c(T-1)] then transpose ----
            HXT = ppool.tile([128, 64], f32, tag="hxt")
            for hf in range(2):
                HX = ppool.tile([64, HB], f32, tag=f"hx{hf}")
                nc.sync.dma_start(HX[0:20, :], rs_hist[(T, hf)][0:20, :])
                nc.sync.dma_start(HX[20:32, :], rs_hist[(T + 1, hf)][20:32, :])
                nc.sync.dma_start(HX[32:64, :], rs_hist[(T + 2, hf)][32:64, :])
                ph_hx = psh_pool.tile([64, 64], f32, tag="ph")
                nc.tensor.transpose(ph_hx[:, :], HX[:], ID64[:])
                # stored state is 2h -> halve on the way out
                nc.vector.tensor_scalar_mul(HXT[64 * hf:64 * hf + 64, :],
                                            ph_hx[:, :], 0.5)

            # ---- outputs ----
            nc.sync.dma_start(actor_d[:], ABUF[:])
            nc.sync.dma_start(value_d[:], VBUF[:])
            nc.sync.dma_start(hx_d[:], HXT[:])

    nc.finalize()
    return nc


_NC_CACHE = None


def _get_nc():
    global _NC_CACHE
    if _NC_CACHE is None:
        _NC_CACHE = _build_program()
    return _NC_CACHE


def kernel(trace=False, **inputs):
    from concourse.bass_utils import run_bass_kernel_spmd

    inp = {k: np.asarray(v, dtype=np.float32) for k, v in inputs.items()}
    WA, WB, WBIAS, BSEL, AW = _build_consts(inp)

    x = inp["x"]          # [B, T, OBS]
    h0 = inp["h0"]        # [B, 64]

    in_maps = []
    for c in range(NCORES):
        b0, b1 = c * BSH, (c + 1) * BSH
        # [T, 2, 64, HB]: per stage, per batch-half, feature-major slice
        xc = x[b0:b1].transpose(1, 2, 0).reshape(T, 64, 2, HB)
        xt = np.ascontiguousarray(xc.transpose(0, 2, 1, 3))
        # kernel state rows hold 2h -> feed doubled initial state
        h0c = (2.0 * h0[b0:b1].T).reshape(64, 2, HB)
        h0t = np.ascontiguousarray(h0c.transpose(1, 0, 2))  # [2, 64, HB]
        in_maps.append({
            "xt": xt, "h0t": h0t, "wa": WA, "wb": WB,
            "wbias": WBIAS, "bsel": BSEL, "aw": AW,
        })

    nc = _get_nc()
    res = run_bass_kernel_spmd(nc, in_maps, list(range(NCORES)), trace=trace)

    actor = np.concatenate([res.results[c]["actor"].reshape(BSH, T, 8)
                            for c in range(NCORES)], axis=0)
    value = np.concatenate([res.results[c]["value"].reshape(BSH, T, 1)
                            for c in range(NCORES)], axis=0)
    hx = np.concatenate([res.results[c]["hx"] for c in range(NCORES)], axis=0)
    kernel.last_exec_time_ns = res.exec_time_ns
    return actor, value, hx


# revision 37
# speedup vs baseline: 1.1727x; 1.0082x over previous
"""CfC (closed-form continuous-time RNN / NCP) kernel for 8 Trainium2 NeuronCores.

Model (see reference): 3 stacked CfC layers, scan over T=256 timesteps,
B=1024 batch, OBS=64 input features; heads: actor [B,T,8], value [B,T,1],
plus final hidden state hx [B,64].

Strategy
--------
Data-parallel over batch: 8 cores x 128 batch rows. Per core the three
layers are computed as a layer-pipelined "wavefront": at stage s the
kernel computes layer0(t=s), layer1(t=s-1), layer2(t=s-2) in the SAME
set of matmuls, because the combined state vector
    S(s) = [n_a(s); n_b(s-1); n_c(s-2)]   (20+12+32 = 64 rows)
feeds all three layers with (overlapping) contiguous K-row windows.

Feature-major layout: activations live as [feature, batch] tiles so the
PE contraction dim (K) is the feature dim. Per stage and per batch-half
(the 128 batch rows are split into two independent 64-row recurrences so
their serial chains interleave across engines):
  rhs RS(s) [128,64] = [S(s-1) (rows 0:64) ; x(s).T (rows 64:128)]
  mm_bias (K=3)  : PSUM[64,192] = biases (start=True broadcast matmul)
  mm_ff1/ff2/t   : PSUM[:,c:c+64] += W.T @ RS  (t-weights half-scaled)
  ACT            : FT = tanh(PSUM)   (sigmoid(a) = (1+tanh(a/2))/2)
  GPSIMD         : d = ff2 - ff1
  DVE            : m = (u+1)*d ; h = 0.5*m + ff1 -> RS(s+1)[0:64]
  GPSIMD         : y = relu(n_c rows of h) -> head staging
  PE (per 3 t's) : head matmul  out[64,27] = Y3.T @ AW  (batch-major)
The head output is batch-major so no transposes are needed except a
single 64x64 PE transpose per half for hx at the end.

Host side transposes x to [T, 2, 64, 64] per core (contiguous per-stage
slices) and gathers/reshapes the outputs.
"""

import numpy as np

OBS, B, T = 64, 1024, 256
SIZES = [(20, 84), (12, 32), (32, 44)]
IN_FEATS = [OBS, 20, 12]
NCORES = 8
BSH = B // NCORES   # 128 batch rows per core
HB = BSH // 2       # 64 batch rows per half
NSTAGES = T + 2     # wavefront ramp: layer2 finishes t=T-1 at stage T+1
NGROUPS = (T + 2) // 3  # 86 head groups of up to 3 timesteps

# S-row layout offsets
H_OFF = [0, 20, 32]       # h-block (recurrent state) offset per layer
IN_OFF = [None, 0, 20]    # input-block offset inside S (layers 1,2)
M_OFF = [0, 20, 32]       # ff1 output row offset per layer (ff2 at +64)


def _build_consts(inp):
    """Build the fused weight matrices (lhsT layouts) on the host.

    DVE tensor_tensor ops require both SBUF operands at the same base
    partition, so ff1 / ff2 / u all live in rows 0:64 of the PSUM/FT tile,
    side by side in columns: cols 0:HB = ff1, HB:2HB = ff2, 2HB:3HB = u.
    """
    f32 = np.float32
    WA = np.zeros((128, 128), f32)   # lhsT: cols 0:64 -> ff1, 64:128 -> ff2
    WB = np.zeros((128, 64), f32)    # lhsT for t preacts (half-scaled)
    bA = np.zeros(128, f32)
    bB = np.zeros(64, f32)
    for l, (h, c) in enumerate(SIZES):
        w1 = inp[f"w1_{l}"] * inp[f"mask_{l}"]
        w2 = inp[f"w2_{l}"] * inp[f"mask_{l}"]
        wt = (inp[f"ta_w_{l}"] + inp[f"tb_w_{l}"]) * 0.5
        nin = IN_FEATS[l]
        mo = M_OFF[l]
        ho = H_OFF[l]
        if l == 0:
            # x part -> K rows 64:128, h part -> K rows 0:20
            WA[64:64 + nin, mo:mo + h] = w1[:, :nin].T
            WA[64:64 + nin, 64 + mo:64 + mo + h] = w2[:, :nin].T
            WA[ho:ho + 20, mo:mo + h] = w1[:, nin:].T
            WA[ho:ho + 20, 64 + mo:64 + mo + h] = w2[:, nin:].T
            WB[64:64 + nin, mo:mo + h] = wt[:, :nin].T
            WB[ho:ho + 20, mo:mo + h] = wt[:, nin:].T
        else:
            io = IN_OFF[l]
            WA[io:io + nin, mo:mo + h] = w1[:, :nin].T
            WA[io:io + nin, 64 + mo:64 + mo + h] = w2[:, :nin].T
            WA[ho:ho + h, mo:mo + h] = w1[:, nin:].T
            WA[ho:ho + h, 64 + mo:64 + mo + h] = w2[:, nin:].T
            WB[io:io + nin, mo:mo + h] = wt[:, :nin].T
            WB[ho:ho + h, mo:mo + h] = wt[:, nin:].T
        bA[mo:mo + h] = inp[f"b1_{l}"]
        bA[64 + mo:64 + mo + h] = inp[f"b2_{l}"]
        bB[mo:mo + h] = (inp[f"ta_b_{l}"] + inp[f"tb_b_{l}"]) * 0.5

    # Doubled-state trick: the kernel stores h' = 2h (so the gating is the
    # constant-free  h' = ff1 + ff2 + u*(ff2-ff1), all plain tensor_tensor
    # ops, which GPSIMD supports).  Consumers compensate: every weight that
    # reads the state rows (K rows 0:64) is halved; x-part rows unchanged.
    WA[0:64, :] *= 0.5
    WB[0:64, :] *= 0.5

    # bias matmul: PSUM[64,3*HB] = WBIAS.T @ BSEL, WBIAS [3,64], BSEL [3,3*HB]
    WBIAS = np.stack([bA[0:64], bA[64:128], bB], axis=0)   # [3, 64]
    BSEL = np.zeros((3, 3 * HB), f32)
    BSEL[0, 0:HB] = 1.0
    BSEL[1, HB:2 * HB] = 1.0
    BSEL[2, 2 * HB:3 * HB] = 1.0

    # head rhs AW [99, 27]: K rows = [y(j=0) 0:32 | y(j=1) 32:64 | y(j=2)
    # 64:96 | ones rows 96:99] -> 3 x (8 actor + 1 value) output cols.
    # (y blocks are 32-partition aligned because GPSIMD requires it.)
    aw, ab = inp["action_w"], inp["action_b"]       # [8,32], [8]
    vw, vb = inp["value_w"], inp["value_b"]         # [1,32], [1]
    AW = np.zeros((99, 27), f32)
    for j in range(3):
        # y rows halved: y3 holds relu(2*n_c) = 2*relu(n_c)
        AW[32 * j:32 * j + 32, 9 * j:9 * j + 8] = aw.T * 0.5
        AW[32 * j:32 * j + 32, 9 * j + 8:9 * j + 9] = vw.T * 0.5
        AW[96 + j, 9 * j:9 * j + 8] = ab
        AW[96 + j, 9 * j + 8] = vb[0]
    return WA, WB, WBIAS, BSEL, AW


def _build_program():
    import concourse.bass as bass
    import concourse.mybir as mybir
    import concourse.tile as tile
    from concourse import bacc
    from concourse.masks import make_identity

    f32 = mybir.dt.float32
    AF = mybir.ActivationFunctionType
    OP = mybir.AluOpType

    nc = bacc.Bacc()

    xt_d = nc.declare_dram_parameter("xt", [T, 2, 64, HB], f32, isOutput=False)
    h0t_d = nc.declare_dram_parameter("h0t", [2, 64, HB], f32, isOutput=False)
    wa_d = nc.declare_dram_parameter("wa", [128, 128], f32, isOutput=False)
    wb_d = nc.declare_dram_parameter("wb", [128, 64], f32, isOutput=False)
    wbias_d = nc.declare_dram_parameter("wbias", [3, 64], f32, isOutput=False)
    bsel_d = nc.declare_dram_parameter("bsel", [3, 3 * HB], f32, isOutput=False)
    aw_d = nc.declare_dram_parameter("aw", [99, 27], f32, isOutput=False)

    actor_d = nc.declare_dram_parameter("actor", [BSH, T * 8], f32, isOutput=True)
    value_d = nc.declare_dram_parameter("value", [BSH, T], f32, isOutput=True)
    hx_d = nc.declare_dram_parameter("hx", [BSH, 64], f32, isOutput=True)

    with tile.TileContext(nc) as tc:
        with (
            tc.tile_pool(name="consts", bufs=1) as cpool,
            tc.tile_pool(name="persist", bufs=1) as ppool,
            tc.tile_pool(name="rs", bufs=10) as rs_pool,
            tc.tile_pool(name="ft", bufs=6) as ft_pool,
            tc.tile_pool(name="dm", bufs=6) as dm_pool,
            tc.tile_pool(name="y3", bufs=4) as y3_pool,
            tc.tile_pool(name="psum", bufs=3, space="PSUM") as ps_pool,
            tc.tile_pool(name="psum_h", bufs=1, space="PSUM") as psh_pool,
        ):
            # ---- constants ----
            WA_t = cpool.tile([128, 128], f32, tag="wa")
            nc.sync.dma_start(WA_t[:], wa_d[:])
            WB_t = cpool.tile([128, 64], f32, tag="wb")
            nc.sync.dma_start(WB_t[:], wb_d[:])
            WBIAS_t = cpool.tile([3, 64], f32, tag="wbias")
            nc.sync.dma_start(WBIAS_t[:], wbias_d[:])
            BSEL_t = cpool.tile([3, 3 * HB], f32, tag="bsel")
            nc.sync.dma_start(BSEL_t[:], bsel_d[:])
            AW_t = cpool.tile([99, 27], f32, tag="aw")
            nc.sync.dma_start(AW_t[:], aw_d[:])
            ID64 = cpool.tile([64, 64], f32, tag="id64")
            make_identity(nc, ID64[:])

            ABUF = ppool.tile([128, T * 8], f32, tag="abuf")
            VBUF = ppool.tile([128, T], f32, tag="vbuf")

            # ---- stage 0 rhs (per half) ----
            rs_cur = []
            for hf in range(2):
                t0 = rs_pool.tile([128, HB], f32, tag=f"rs{hf}")
                nc.sync.dma_start(t0[0:64, :], h0t_d[hf])
                nc.sync.dma_start(t0[64:128, :], xt_d[0, hf])
                rs_cur.append(t0)

            y3 = [None, None]
            rs_hist = {}  # (stage, hf) handles for the hx assembly

            for s in range(NSTAGES):
                rs_next = []
                for hf in range(2):
                    rn = rs_pool.tile([128, HB], f32, tag=f"rs{hf}")
                    if s + 1 < T:
                        nc.sync.dma_start(rn[64:128, :], xt_d[s + 1, hf])
                    elif s + 1 <= NSTAGES:
                        nc.gpsimd.memset(rn[64:96, :], 0.0)
                        nc.gpsimd.memset(rn[96:128, :], 0.0)
                    rs_next.append(rn)

                for hf in range(2):
                    rsc, rsn = rs_cur[hf], rs_next[hf]
                    # matmuls: bias broadcast then accumulate ff1 / ff2 / t
                    pab = ps_pool.tile([64, 3 * HB], f32, tag=f"pab{hf}")
                    nc.tensor.matmul(pab[:, :], WBIAS_t[:], BSEL_t[:],
                                     start=True, stop=False,
                                     skip_group_check=True)
                    nc.tensor.matmul(pab[:, 0:HB], WA_t[:, 0:64], rsc[:],
                                     start=False, stop=True,
                                     skip_group_check=True)
                    nc.tensor.matmul(pab[:, HB:2 * HB], WA_t[:, 64:128],
                                     rsc[:], start=False, stop=True,
                                     skip_group_check=True)
                    nc.tensor.matmul(pab[:, 2 * HB:3 * HB], WB_t[:], rsc[:],
                                     start=False, stop=True,
                                     skip_group_check=True)

                    # merged tanh: ff1 | ff2 | u (all at base partition 0)
                    ft = ft_pool.tile([64, 3 * HB], f32, tag=f"ft{hf}")
                    nc.scalar.activation(ft[:, :], pab[:, :], AF.Tanh)

                    # gating in doubled-state form (all plain TT, all on
                    # GPSIMD so the dependent ops pipeline on one engine):
                    #   h' = 2h = ff1 + ff2 + u*(ff2-ff1)
                    d_t = dm_pool.tile([64, HB], f32, tag=f"d{hf}")
                    nc.gpsimd.tensor_sub(d_t[:], ft[:, HB:2 * HB],
                                         ft[:, 0:HB])
                    s_t = dm_pool.tile([64, HB], f32, tag=f"s{hf}")
                    nc.gpsimd.tensor_add(s_t[:], ft[:, 0:HB],
                                         ft[:, HB:2 * HB])
                    p_t = dm_pool.tile([64, HB], f32, tag=f"p{hf}")
                    nc.gpsimd.tensor_mul(p_t[:], ft[:, 2 * HB:3 * HB],
                                         d_t[:])
                    nc.gpsimd.tensor_add(rsn[0:64, :], s_t[:], p_t[:])

                    # wavefront ramp: restore h0 for layers not yet active
                    # (DMA: engine copies need 32-aligned partition bases)
                    if s == 0:
                        nc.sync.dma_start(rsn[20:64, :], h0t_d[hf, 20:64, :])
                    elif s == 1:
                        nc.sync.dma_start(rsn[32:64, :], h0t_d[hf, 32:64, :])

                    # head: y = relu(n_c) for t_out = s-2; both halves share
                    # one Y3 [99,128] (halves side by side in columns) so a
                    # single head matmul covers the full 128-row batch
                    if s >= 2:
                        t_out = s - 2
                        j = t_out % 3
                        g = t_out // 3
                        if j == 0 and hf == 0:
                            y3[0] = y3_pool.tile([99, BSH], f32,
                                                 name="y3g", tag="y3")
                            nc.gpsimd.memset(y3[0][96:99, :], 1.0)
                            if t_out == T - 1:
                                # tail group: zero unused y(j=1,2) slots
                                nc.gpsimd.memset(y3[0][32:64, :], 0.0)
                                nc.gpsimd.memset(y3[0][64:96, :], 0.0)
                        # relu on DVE (Pool carries the gating chain);
                        # y3 holds 2*relu(n_c), AW y-rows are pre-halved
                        nc.vector.tensor_scalar_max(
                            y3[0][32 * j:32 * j + 32, HB * hf:HB * hf + HB],
                            rsn[32:64, :], 0.0)
                        if (j == 2 or t_out == T - 1) and hf == 1:
                            ph = psh_pool.tile([128, 27], f32, tag="ph")
                            nc.tensor.matmul(ph[:, :], y3[0][:, :],
                                             AW_t[:, :], start=True,
                                             stop=True)
                            # scatter straight into the output buffers:
                            # ph cols = [a0(8) v0 a1(8) v1 a2(8) v2]
                            t0 = 3 * g
                            ph3 = ph[:, 0:27].rearrange(
                                "p (jj q) -> p jj q", q=9)
                            if t_out == T - 1:
                                # tail group holds only t = 255
                                nc.vector.tensor_copy(
                                    ABUF[:, 8 * t0:8 * t0 + 8],
                                    ph[:, 0:8])
                                nc.vector.tensor_copy(
                                    VBUF[:, t0:t0 + 1], ph[:, 8:9])
                            else:
                                dst_a = ABUF[:, 8 * t0:8 * t0 + 24]
                                dst_a = dst_a.rearrange(
                                    "p (jj o) -> p jj o", o=8)
                                nc.vector.tensor_copy(dst_a,
                                                      ph3[:, :, 0:8])
                                dst_v = VBUF[:, t0:t0 + 3].rearrange(
                                    "p (jj q) -> p jj q", q=1)
                                nc.vector.tensor_copy(dst_v,
                                                      ph3[:, :, 8:9])
                            # stream the actor output out in chunks
                            if g in (20, 41, 62):
                                c0 = 0 if g == 20 else 8 * 3 * (g - 20)
                                c1 = 8 * 3 * (g + 1)
                                nc.sync.dma_start(actor_d[:, c0:c1],
                                                  ABUF[:, c0:c1])

                    if s >= NSTAGES - 3:
                        rs_hist[(s + 1, hf)] = rs_next[hf]
                rs_cur = rs_next

            # ---- hx: [n_a(T-1); n_b(T-1); n_c(T-1)] then transpose ----
            HXT = ppool.tile([128, 64], f32, tag="hxt")
            for hf in range(2):
                HX = ppool.tile([64, HB], f32, tag=f"hx{hf}")
                nc.sync.dma_start(HX[0:20, :], rs_hist[(T, hf)][0:20, :])
                nc.sync.dma_start(HX[20:32, :], rs_hist[(T + 1, hf)][20:32, :])
                nc.sync.dma_start(HX[32:64, :], rs_hist[(T + 2, hf)][32:64, :])
                ph_hx = psh_pool.tile([64, 64], f32, tag="ph")
                nc.tensor.transpose(ph_hx[:, :], HX[:], ID64[:])
                # stored state is 2h -> halve on the way out
                nc.vector.tensor_scalar_mul(HXT[64 * hf:64 * hf + 64, :],
                                            ph_hx[:, :], 0.5)

            # ---- outputs (actor cols 0:1512 already streamed in-loop) ----
            nc.sync.dma_start(actor_d[:, 1512:2048], ABUF[:, 1512:2048])
            nc.sync.dma_start(value_d[:], VBUF[:])
            nc.sync.dma_start(hx_d[:], HXT[:])

    nc.finalize()
    return nc


_NC_CACHE = None


def _get_nc():
    global _NC_CACHE
    if _NC_CACHE is None:
        _NC_CACHE = _build_program()
    return _NC_CACHE


def kernel(trace=False, **inputs):
    from concourse.bass_utils import run_bass_kernel_spmd

    inp = {k: np.asarray(v, dtype=np.float32) for k, v in inputs.items()}
    WA, WB, WBIAS, BSEL, AW = _build_consts(inp)

    x = inp["x"]          # [B, T, OBS]
    h0 = inp["h0"]        # [B, 64]

    in_maps = []
    for c in range(NCORES):
        b0, b1 = c * BSH, (c + 1) * BSH
        # [T, 2, 64, HB]: per stage, per batch-half, feature-major slice
        xc = x[b0:b1].transpose(1, 2, 0).reshape(T, 64, 2, HB)
        xt = np.ascontiguousarray(xc.transpose(0, 2, 1, 3))
        # kernel state rows hold 2h -> feed doubled initial state
        h0c = (2.0 * h0[b0:b1].T).reshape(64, 2, HB)
        h0t = np.ascontiguousarray(h0c.transpose(1, 0, 2))  # [2, 64, HB]
        in_maps.append({
            "xt": xt, "h0t": h0t, "wa": WA, "wb": WB,
            "wbias": WBIAS, "bsel": BSEL, "aw": AW,
        })

    nc = _get_nc()
    res = run_bass_kernel_spmd(nc, in_maps, list(range(NCORES)), trace=trace)

    actor = np.concatenate([res.results[c]["actor"].reshape(BSH, T, 8)
                            for c in range(NCORES)], axis=0)
    value = np.concatenate([res.results[c]["value"].reshape(BSH, T, 1)
                            for c in range(NCORES)], axis=0)
    hx = np.concatenate([res.results[c]["hx"] for c in range(NCORES)], axis=0)
    kernel.last_exec_time_ns = res.exec_time_ns
    return actor, value, hx
